# revision 52
# baseline (speedup 1.0000x reference)
"""Depthwise causal Conv1d (k=4) + SiLU on 8 Trainium2 NeuronCores.

Problem: x [4, 4096, 2048] f32, w [2048, 4] f32,
out[b, t, d] = silu(sum_j w[d, j] * x[b, t - 3 + j, d])   (zero-padded left).

Sharding: 8 cores = 4 batches x 2 channel-halves. Depthwise conv is
independent per channel, so channel sharding needs no halo exchange.

Layout: each core receives its shard host-transposed to [channels, time]
(channels on SBUF partitions). The per-channel weight w[d, j] is then a
per-partition scalar and the causal time shifts are free-dim AP offsets
into one loaded tile.

Precision: x and the output are host-cast fp16 (halves HBM traffic both
ways); products and the add tree stay fp16 (PE accumulates fp32 in
PSUM); SiLU computes fp32-internally on ACT. Rel err ~5e-4.

Schedule (DMA-bound problem: ~16.8 MB/core over 16 DMA engines, each
~26 GB/s; measured floor ~44 us/engine of transfer work):
 - All 8 channel-block rows of x are loaded up-front into SBUF (fits:
   ~66 KB/partition), ordered to match consumption, with blocks 0/1
   arriving in quarter/half pieces so both compute engines ramp by
   ~11 us. Loads issue on SyncE (HWDGE).
 - 5 blocks run on the TensorEngine as diag(w_j) matmuls accumulating
   the 4 taps in 1024-col PSUM tiles (LDWEIGHTS pipelines under the
   previous matmul; ~97% of peak fp16 rate); 3 blocks run on DVE as 4
   tensor_scalar products (1-src ops hit the 2x fp16 mode, ~3 elem/ns)
   + 3 plain non-aliased adds (2-src ops run ~1.8 elem/ns; the fused
   scalar_tensor_tensor measured slower at ~0.9 elem/ns, and GpSimd
   elementwise is microcode-slow - never use either for bulk work).
 - ACT does SiLU only, in 1024-col pieces (PE chunks straight out of
   PSUM): uniform granularity keeps its in-order queue from
   head-of-line blocking the PE path's PSUM drain. The chunk stream is
   ordered by estimated completion time (DVE ~3 us per 1024 cols vs PE
   ~1.9 us) so the SiLU queue matches production order.
 - Stores issue on GpSimd (SWDGE): separate rings from the HWDGE load
   rings, so DMA engines round-robin loads and stores instead of
   draining the whole load backlog first. The last few stores issue on
   ACT (HWDGE, program-order adjacent to their SiLU) to cut the drain.

Measured ~60-64 us on hardware (baseline 72 us); PE/DVE/ACT all run at
their throughput ceilings with DMA ~88% packed.
"""

import sys
import types

import numpy as np

import concourse.bass as bass
import concourse.bacc as bacc
import concourse.mybir as mybir
from concourse.tile import TileContext
from concourse.bass_utils import run_bass_kernel_spmd


def _ensure_ntff_hook():
    """bass_utils imports antenv.axon_hooks when BASS_TRACE is set; that
    module is absent on this image. Install a shim so tracing works when
    possible and degrades gracefully (instead of crashing) when not."""
    try:
        import antenv.axon_hooks  # noqa: F401

        return
    except ImportError:
        pass
    try:
        import antenv

        hook = None
        try:
            if "/root/.axon_site" not in sys.path:
                sys.path.insert(0, "/root/.axon_site")
            from trn_agent_boot.trn_boot import _ntff_profile_via_ctypes

            hook = _ntff_profile_via_ctypes("/opt/axon/libaxon_pjrt.so")
        except Exception:
            hook = None
        mod = types.ModuleType("antenv.axon_hooks")
        mod._hook = hook
        mod.get_axon_ntff_profile_hook = lambda: mod._hook
        mod.set_axon_ntff_profile_hook = lambda h: setattr(mod, "_hook", h)
        sys.modules["antenv.axon_hooks"] = mod
        antenv.axon_hooks = mod
    except Exception:
        pass


_ensure_ntff_hook()

B, L, D = 4, 4096, 2048
K = 4
PAD = K - 1
N_CORES = 8
DH = D // 2            # channels per core
NBLK = DH // 128       # 128-partition channel blocks per core
ROWW = 4128            # DRAM row stride (fp16 elems): 64B-aligned rows

MID_DT = mybir.dt.float16
D_BLKS = [0, 2, 4]     # DVE vector-path blocks
P_BLKS = [1, 3, 5, 6, 7]  # TensorEngine (diag matmul) blocks
CH = 2048              # compute/store chunk (cols)

_cache = {}


def _build_bass():
    nc = bacc.Bacc()
    xt = nc.dram_tensor("xt", [DH, ROWW], MID_DT, kind="ExternalInput")
    wt = nc.dram_tensor("wt", [128, NBLK * K], mybir.dt.float32, kind="ExternalInput")
    # diag(w) blocks for the PE path, packed for P_BLKS only: partition p,
    # col (pi*K + j)*128 + m holds w[P_BLKS[pi]*128 + p, j] iff m == p else 0
    wd = nc.dram_tensor(
        "wd", [128, len(P_BLKS) * K * 128], MID_DT, kind="ExternalInput"
    )
    ot = nc.dram_tensor("ot", [DH, L], MID_DT, kind="ExternalOutput")
    f32 = mybir.dt.float32

    with TileContext(nc) as tc:
        with tc.tile_pool(name="pool", bufs=2) as pool, \
             tc.tile_pool(name="psum", bufs=2, space="PSUM") as psum_pool:
            w = pool.tile([128, NBLK * K], f32, tag="w", bufs=1)
            wdt = pool.tile([128, len(P_BLKS) * K * 128], MID_DT, tag="wd", bufs=1)
            xtile = {}
            # blocks 0 and 1 load a small first quarter, and the first PE
            # block's diag slice loads separately, so both compute engines
            # start as early as possible.
            Q = 1024
            x0 = pool.tile([128, L + PAD + 1], MID_DT, tag="x0", bufs=1)
            x1 = pool.tile([128, L + PAD + 1], MID_DT, tag="x1", bufs=1)
            xtile[0], xtile[1] = x0, x1
            for blk in range(2, NBLK):
                xb = pool.tile([128, L + PAD + 1], MID_DT, tag=f"x{blk}", bufs=1)
                xtile[blk] = xb

            def load_x(blk, c0, c1):
                nc.sync.dma_start(
                    out=xtile[blk][:, c0:c1],
                    in_=xt[blk * 128 : (blk + 1) * 128, c0:c1],
                )

            # loads ordered to match consumption. The PE path's ramp
            # dependencies (block-1 diag slice + x1's first quarter) issue
            # on ACT's HWDGE rings IN PARALLEL with SyncE's x0 pieces, so
            # both compute engines' deps land ~9.5 us and neither start
            # serializes behind the other's loads.
            nc.scalar.dma_start(out=wdt[:, 0 : K * 128], in_=wd[:, 0 : K * 128])
            nc.scalar.dma_start(
                out=xtile[1][:, 0 : 512 + PAD], in_=xt[128:256, 0 : 512 + PAD]
            )
            nc.scalar.dma_start(
                out=xtile[1][:, 512 + PAD : Q + PAD],
                in_=xt[128:256, 512 + PAD : Q + PAD],
            )
            nc.scalar.dma_start(
                out=wdt[:, K * 128 : 2 * K * 128], in_=wd[:, K * 128 : 2 * K * 128]
            )
            # Warmup: a tiny Silu forces the silu activation-table set to
            # load during the initial DMA wait; it is the only table load
            # in the whole kernel.
            warm = pool.tile([128, 2], MID_DT, tag="warm", bufs=1)
            nc.vector.memset(warm[:], 0.0)
            nc.scalar.activation(warm[:], warm[:], mybir.ActivationFunctionType.Silu)

            load_x(0, 0, Q + PAD)
            nc.sync.dma_start(out=w[:], in_=wt[:, :])
            load_x(0, Q + PAD, L + PAD)
            load_x(1, Q + PAD, CH + PAD)
            load_x(1, CH + PAD, L + PAD)
            load_x(2, 0, CH + PAD)
            load_x(3, 0, L + PAD)
            nc.sync.dma_start(out=wdt[:, 2 * K * 128 :], in_=wd[:, 2 * K * 128 :])
            load_x(2, CH + PAD, L + PAD)
            for blk in [5, 4, 6, 7]:
                load_x(blk, 0, L + PAD)

            def _store(o, blk, s0, slen, store_eng):
                r0 = blk * 128
                eng = store_eng or nc.gpsimd
                eng.dma_start(
                    out=ot[r0 : r0 + 128, s0 : s0 + slen], in_=o[:, 0:slen]
                )

            def d_unit(blk, t0, tl, store_eng=None, o=None, o_off=0, flush=True):
                # products shift-rebased (m_j[:, t] = w_j * x[:, t + j]) so
                # the fp16 add tree stays aligned; plain non-aliased adds.
                x = xtile[blk]
                wj = lambda j: w[:, blk * K + j : blk * K + j + 1]
                m = pool.tile([128, 4, CH], MID_DT, tag="m", bufs=2)
                for j in range(K):
                    nc.vector.tensor_scalar_mul(
                        m[:, j, 0:tl], x[:, t0 + j : t0 + j + tl], wj(j)
                    )
                a = pool.tile([128, 3, CH], MID_DT, tag="a", bufs=4)
                nc.vector.tensor_add(a[:, 0, 0:tl], m[:, 0, 0:tl], m[:, 1, 0:tl])
                nc.vector.tensor_add(a[:, 1, 0:tl], m[:, 2, 0:tl], m[:, 3, 0:tl])
                nc.vector.tensor_add(a[:, 2, 0:tl], a[:, 0, 0:tl], a[:, 1, 0:tl])
                if o is None:
                    o = pool.tile([128, CH], MID_DT, tag="o", bufs=5)
                # silu in 1024-col pieces: uniform ACT granularity caps
                # head-of-line blocking of the PE path's PSUM drain
                for h0 in range(0, tl, 1024):
                    pl = min(1024, tl - h0)
                    nc.scalar.activation(
                        o[:, o_off + h0 : o_off + h0 + pl],
                        a[:, 2, h0 : h0 + pl],
                        mybir.ActivationFunctionType.Silu,
                    )
                if flush:
                    _store(o, blk, t0 + tl - (o_off + tl), o_off + tl, store_eng)
                return o

            def p_unit(blk, t0, tl, store_eng=None, o=None, o_off=0, flush=True):
                x = xtile[blk]
                pi = P_BLKS.index(blk)
                # 1024-col PSUM tiles (2 banks) x 4 bufs: PE runs up to 4
                # tiles ahead of ACT's PSUM drain.
                if o is None:
                    o = pool.tile([128, CH], MID_DT, tag="o", bufs=5)
                for h0 in range(0, tl, 1024):
                    pl = min(1024, tl - h0)
                    ps = psum_pool.tile([128, 1024], f32, tag="ps", bufs=4)
                    for c in range(pl // 512):
                        for j in range(K):
                            lw = wdt[:, (pi * K + j) * 128 : (pi * K + j + 1) * 128]
                            b0 = t0 + h0 + c * 512
                            nc.tensor.matmul(
                                ps[:, c * 512 : (c + 1) * 512],
                                lw,
                                x[:, b0 + j : b0 + j + 512],
                                start=(j == 0),
                                stop=(j == K - 1),
                            )
                    nc.scalar.activation(
                        o[:, o_off + h0 : o_off + h0 + pl],
                        ps[:, 0:pl],
                        mybir.ActivationFunctionType.Silu,
                    )
                if flush:
                    _store(o, blk, t0 + tl - (o_off + tl), o_off + tl, store_eng)
                return o

            # chunk stream ordered by estimated completion time (DVE ~3 us
            # per 1024 cols, PE ~1.85 us) so ACT's in-order SiLU queue
            # matches production order and neither engine head-of-line
            # blocks the other's drain. Blocks 0/1 start with 1024-col
            # units so both engines ramp sooner. Stores issue on GpSimd
            # (SWDGE) except the very last, which rides ACT HWDGE
            # (program-order adjacent to its SiLU, shortening the drain).
            act = nc.scalar
            # blocks 0/1 ramp with 1024-col units sharing one o tile per
            # 2048-col half (single merged store - fewer completion
            # semaphores for the end barrier to poll).
            # PE's very first unit is 512 cols (PE per-op overhead is
            # negligible, so the earlier start shifts its whole packed
            # stream - and the kernel end - ~0.8 us earlier)
            o1 = p_unit(1, 0, 512, flush=False)
            p_unit(1, 512, 512, o=o1, o_off=512, flush=False)
            o0 = d_unit(0, 0, Q, flush=False)
            p_unit(1, Q, Q, o=o1, o_off=Q)
            d_unit(0, Q, Q, o=o0, o_off=Q)
            # blocks 2-6 share one [128, 2*CH] o tile per block: a single
            # 4096-col store each (8KB descriptors, and fewer completion
            # semaphores for the end barrier to poll serially).
            obt = {}

            def first_half(fn, blk):
                ob = pool.tile([128, 2 * CH], MID_DT, tag="ob", bufs=3)
                obt[blk] = ob
                fn(blk, 0, CH, None, o=ob, flush=False)

            def second_half(fn, blk):
                fn(blk, CH, CH, None, o=obt[blk], o_off=CH)

            stream = [
                (p_unit, 1, CH, CH, None), (d_unit, 0, CH, CH, None),
                (first_half, p_unit, 3), (second_half, p_unit, 3),
                (first_half, d_unit, 2), (first_half, p_unit, 5),
                (second_half, p_unit, 5), (second_half, d_unit, 2),
                (first_half, p_unit, 6), (first_half, d_unit, 4),
                (second_half, p_unit, 6), (p_unit, 7, 0, CH, None),
                (second_half, d_unit, 4), (p_unit, 7, CH, Q, None),
                (p_unit, 7, CH + Q, Q, act),
            ]
            for entry in stream:
                if entry[0] in (first_half, second_half):
                    entry[0](entry[1], entry[2])
                else:
                    fn, blk, t0, tl, se = entry
                    fn(blk, t0, tl, se)
    nc.compile()
    return nc


def _shard_inputs(x, w):
    in_maps = []
    for core in range(N_CORES):
        b, half = divmod(core, 2)
        d0 = half * DH
        xt = np.zeros((DH, ROWW), dtype=np.float16)
        xt[:, PAD : PAD + L] = x[b, :, d0 : d0 + DH].T.astype(np.float16)
        # w rows for this shard, rearranged so partition p holds the K
        # weights of channel blk*128 + p at free cols [blk*K, blk*K + K)
        w_sh = w[d0 : d0 + DH].reshape(NBLK, 128, K)
        wt = (
            w_sh.transpose(1, 0, 2).reshape(128, NBLK * K).astype(np.float32)
        )
        # diag blocks for the PE path (P_BLKS only)
        wdv = np.zeros((128, len(P_BLKS), K, 128), dtype=np.float16)
        idx = np.arange(128)
        wdv[idx, :, :, idx] = w_sh[P_BLKS].transpose(1, 0, 2).astype(np.float16)
        in_maps.append(
            {
                "xt": np.ascontiguousarray(xt),
                "wt": np.ascontiguousarray(wt),
                "wd": np.ascontiguousarray(
                    wdv.reshape(128, len(P_BLKS) * K * 128)
                ),
            }
        )
    return in_maps


def kernel(x, w):
    x = np.asarray(x, dtype=np.float32)
    w = np.asarray(w, dtype=np.float32)
    assert x.shape == (B, L, D) and w.shape == (D, K)

    if "nc" not in _cache:
        _cache["nc"] = _build_bass()
    nc = _cache["nc"]

    in_maps = _shard_inputs(x, w)
    res = None
    for attempt in range(3):
        try:
            res = run_bass_kernel_spmd(nc, in_maps, core_ids=list(range(N_CORES)))
            break
        except Exception:
            if attempt == 2:
                raise
    _cache["last_results"] = res

    out = np.empty((B, L, D), dtype=np.float32)
    for core in range(N_CORES):
        b, half = divmod(core, 2)
        d0 = half * DH
        out[b, :, d0 : d0 + DH] = res.results[core]["ot"].T.astype(np.float32)
    return out


# revision 53
# speedup vs baseline: 1.1296x; 1.1296x over previous
"""Depthwise causal Conv1d (k=4) + SiLU on 8 Trainium2 NeuronCores.

Problem: x [4, 4096, 2048] f32, w [2048, 4] f32,
out[b, t, d] = silu(sum_j w[d, j] * x[b, t - 3 + j, d])   (zero-padded left).

Sharding: 8 cores = 4 batches x 2 channel-halves. Depthwise conv is
independent per channel, so channel sharding needs no halo exchange.

Layout: each core receives its shard host-transposed to [channels, time]
(channels on SBUF partitions). The per-channel weight w[d, j] is then a
per-partition scalar and the causal time shifts are free-dim AP offsets
into one loaded tile.

Precision: x and the output are host-cast fp16 (halves HBM traffic both
ways); products and the add tree stay fp16 (PE accumulates fp32 in
PSUM); SiLU computes fp32-internally on ACT. Rel err ~5e-4.

Schedule (DMA-bound problem: ~16.8 MB/core over 16 DMA engines, each
~26 GB/s; measured floor ~44 us/engine of transfer work):
 - All 8 channel-block rows of x are loaded up-front into SBUF (fits:
   ~66 KB/partition), ordered to match consumption, with blocks 0/1
   arriving in quarter/half pieces so both compute engines ramp by
   ~11 us. Loads issue on SyncE (HWDGE).
 - 5 blocks run on the TensorEngine as diag(w_j) matmuls accumulating
   the 4 taps in 1024-col PSUM tiles (LDWEIGHTS pipelines under the
   previous matmul; ~97% of peak fp16 rate); 3 blocks run on DVE as 4
   tensor_scalar products (1-src ops hit the 2x fp16 mode, ~3 elem/ns)
   + 3 plain non-aliased adds (2-src ops run ~1.8 elem/ns; the fused
   scalar_tensor_tensor measured slower at ~0.9 elem/ns, and GpSimd
   elementwise is microcode-slow - never use either for bulk work).
 - ACT does SiLU only, in 1024-col pieces (PE chunks straight out of
   PSUM): uniform granularity keeps its in-order queue from
   head-of-line blocking the PE path's PSUM drain. The chunk stream is
   ordered by estimated completion time (DVE ~3 us per 1024 cols vs PE
   ~1.9 us) so the SiLU queue matches production order.
 - Stores issue on GpSimd (SWDGE): separate rings from the HWDGE load
   rings, so DMA engines round-robin loads and stores instead of
   draining the whole load backlog first. The last few stores issue on
   ACT (HWDGE, program-order adjacent to their SiLU) to cut the drain.

Measured ~60-64 us on hardware (baseline 72 us); PE/DVE/ACT all run at
their throughput ceilings with DMA ~88% packed.
"""

import sys
import types

import numpy as np

import concourse.bass as bass
import concourse.bacc as bacc
import concourse.mybir as mybir
from concourse.tile import TileContext
from concourse.bass_utils import run_bass_kernel_spmd


def _ensure_ntff_hook():
    """bass_utils imports antenv.axon_hooks when BASS_TRACE is set; that
    module is absent on this image. Install a shim so tracing works when
    possible and degrades gracefully (instead of crashing) when not."""
    try:
        import antenv.axon_hooks  # noqa: F401

        return
    except ImportError:
        pass
    try:
        import antenv

        hook = None
        try:
            if "/root/.axon_site" not in sys.path:
                sys.path.insert(0, "/root/.axon_site")
            from trn_agent_boot.trn_boot import _ntff_profile_via_ctypes

            hook = _ntff_profile_via_ctypes("/opt/axon/libaxon_pjrt.so")
        except Exception:
            hook = None
        mod = types.ModuleType("antenv.axon_hooks")
        mod._hook = hook
        mod.get_axon_ntff_profile_hook = lambda: mod._hook
        mod.set_axon_ntff_profile_hook = lambda h: setattr(mod, "_hook", h)
        sys.modules["antenv.axon_hooks"] = mod
        antenv.axon_hooks = mod
    except Exception:
        pass


_ensure_ntff_hook()

B, L, D = 4, 4096, 2048
K = 4
PAD = K - 1
N_CORES = 8
DH = D // 2            # channels per core
NBLK = DH // 128       # 128-partition channel blocks per core
ROWW = 4128            # DRAM row stride (fp16 elems): 64B-aligned rows

MID_DT = mybir.dt.float16
D_BLKS = [0, 2, 4]     # DVE vector-path blocks
P_BLKS = [1, 3, 5, 6, 7]  # TensorEngine (diag matmul) blocks
CH = 2048              # compute/store chunk (cols)

_cache = {}


def _build_bass():
    nc = bacc.Bacc()
    xt = nc.dram_tensor("xt", [DH, ROWW], MID_DT, kind="ExternalInput")
    wt = nc.dram_tensor("wt", [128, NBLK * K], mybir.dt.float32, kind="ExternalInput")
    # diag(w) blocks for the PE path, packed for P_BLKS only: partition p,
    # col (pi*K + j)*128 + m holds w[P_BLKS[pi]*128 + p, j] iff m == p else 0
    wd = nc.dram_tensor(
        "wd", [128, len(P_BLKS) * K * 128], MID_DT, kind="ExternalInput"
    )
    ot = nc.dram_tensor("ot", [DH, L], MID_DT, kind="ExternalOutput")
    f32 = mybir.dt.float32

    with TileContext(nc) as tc:
        with tc.tile_pool(name="pool", bufs=2) as pool, \
             tc.tile_pool(name="psum", bufs=2, space="PSUM") as psum_pool:
            w = pool.tile([128, NBLK * K], f32, tag="w", bufs=1)
            wdt = pool.tile([128, len(P_BLKS) * K * 128], MID_DT, tag="wd", bufs=1)
            xtile = {}
            # blocks 0 and 1 load a small first quarter, and the first PE
            # block's diag slice loads separately, so both compute engines
            # start as early as possible.
            Q = 1024
            x0 = pool.tile([128, L + PAD + 1], MID_DT, tag="x0", bufs=1)
            x1 = pool.tile([128, L + PAD + 1], MID_DT, tag="x1", bufs=1)
            xtile[0], xtile[1] = x0, x1
            for blk in range(2, NBLK):
                xb = pool.tile([128, L + PAD + 1], MID_DT, tag=f"x{blk}", bufs=1)
                xtile[blk] = xb

            def load_x(blk, c0, c1):
                nc.sync.dma_start(
                    out=xtile[blk][:, c0:c1],
                    in_=xt[blk * 128 : (blk + 1) * 128, c0:c1],
                )

            # loads ordered to match consumption. The PE path's ramp
            # dependencies (block-1 diag slice + x1's first quarter) issue
            # on ACT's HWDGE rings IN PARALLEL with SyncE's x0 pieces, so
            # both compute engines' deps land ~9.5 us and neither start
            # serializes behind the other's loads.
            nc.scalar.dma_start(out=wdt[:, 0 : K * 128], in_=wd[:, 0 : K * 128])
            nc.scalar.dma_start(
                out=xtile[1][:, 0 : 512 + PAD], in_=xt[128:256, 0 : 512 + PAD]
            )
            nc.scalar.dma_start(
                out=xtile[1][:, 512 + PAD : Q + PAD],
                in_=xt[128:256, 512 + PAD : Q + PAD],
            )
            nc.scalar.dma_start(
                out=wdt[:, K * 128 : 2 * K * 128], in_=wd[:, K * 128 : 2 * K * 128]
            )
            # Warmup: a tiny Silu forces the silu activation-table set to
            # load during the initial DMA wait; it is the only table load
            # in the whole kernel.
            warm = pool.tile([128, 2], MID_DT, tag="warm", bufs=1)
            nc.vector.memset(warm[:], 0.0)
            nc.scalar.activation(warm[:], warm[:], mybir.ActivationFunctionType.Silu)

            load_x(0, 0, Q + PAD)
            nc.sync.dma_start(out=w[:], in_=wt[:, :])
            load_x(0, Q + PAD, CH + PAD)
            load_x(1, Q + PAD, CH + PAD)
            load_x(0, CH + PAD, L + PAD)
            load_x(1, CH + PAD, L + PAD)
            load_x(2, 0, CH + PAD)
            load_x(3, 0, L + PAD)
            nc.sync.dma_start(out=wdt[:, 2 * K * 128 :], in_=wd[:, 2 * K * 128 :])
            load_x(2, CH + PAD, L + PAD)
            for blk in [5, 4, 6, 7]:
                load_x(blk, 0, L + PAD)

            def _store(o, blk, s0, slen, store_eng):
                r0 = blk * 128
                eng = store_eng or nc.gpsimd
                eng.dma_start(
                    out=ot[r0 : r0 + 128, s0 : s0 + slen], in_=o[:, 0:slen]
                )

            def d_unit(blk, t0, tl, store_eng=None, o=None, o_off=0, flush=True):
                # products shift-rebased (m_j[:, t] = w_j * x[:, t + j]) so
                # the fp16 add tree stays aligned; plain non-aliased adds.
                x = xtile[blk]
                wj = lambda j: w[:, blk * K + j : blk * K + j + 1]
                m = pool.tile([128, 4, CH], MID_DT, tag="m", bufs=2)
                for j in range(K):
                    nc.vector.tensor_scalar_mul(
                        m[:, j, 0:tl], x[:, t0 + j : t0 + j + tl], wj(j)
                    )
                a = pool.tile([128, 3, CH], MID_DT, tag="a", bufs=4)
                nc.vector.tensor_add(a[:, 0, 0:tl], m[:, 0, 0:tl], m[:, 1, 0:tl])
                nc.vector.tensor_add(a[:, 1, 0:tl], m[:, 2, 0:tl], m[:, 3, 0:tl])
                nc.vector.tensor_add(a[:, 2, 0:tl], a[:, 0, 0:tl], a[:, 1, 0:tl])
                if o is None:
                    o = pool.tile([128, CH], MID_DT, tag="o", bufs=9)
                # silu in 1024-col pieces: uniform ACT granularity caps
                # head-of-line blocking of the PE path's PSUM drain
                for h0 in range(0, tl, 1024):
                    pl = min(1024, tl - h0)
                    nc.scalar.activation(
                        o[:, o_off + h0 : o_off + h0 + pl],
                        a[:, 2, h0 : h0 + pl],
                        mybir.ActivationFunctionType.Silu,
                    )
                if flush:
                    _store(o, blk, t0 + tl - (o_off + tl), o_off + tl, store_eng)
                return o

            def p_unit(blk, t0, tl, store_eng=None, o=None, o_off=0, flush=True):
                x = xtile[blk]
                pi = P_BLKS.index(blk)
                # 1024-col PSUM tiles (2 banks) x 4 bufs: PE runs up to 4
                # tiles ahead of ACT's PSUM drain.
                if o is None:
                    o = pool.tile([128, CH], MID_DT, tag="o", bufs=9)
                for h0 in range(0, tl, 1024):
                    pl = min(1024, tl - h0)
                    ps = psum_pool.tile([128, 1024], f32, tag="ps", bufs=4)
                    for c in range(pl // 512):
                        for j in range(K):
                            lw = wdt[:, (pi * K + j) * 128 : (pi * K + j + 1) * 128]
                            b0 = t0 + h0 + c * 512
                            nc.tensor.matmul(
                                ps[:, c * 512 : (c + 1) * 512],
                                lw,
                                x[:, b0 + j : b0 + j + 512],
                                start=(j == 0),
                                stop=(j == K - 1),
                            )
                    nc.scalar.activation(
                        o[:, o_off + h0 : o_off + h0 + pl],
                        ps[:, 0:pl],
                        mybir.ActivationFunctionType.Silu,
                    )
                if flush:
                    _store(o, blk, t0 + tl - (o_off + tl), o_off + tl, store_eng)
                return o

            # chunk stream ordered by estimated completion time (DVE ~3 us
            # per 1024 cols, PE ~1.85 us) so ACT's in-order SiLU queue
            # matches production order and neither engine head-of-line
            # blocks the other's drain. Blocks 0/1 start with 1024-col
            # units so both engines ramp sooner. Stores issue on GpSimd
            # (SWDGE) except the very last, which rides ACT HWDGE
            # (program-order adjacent to its SiLU, shortening the drain).
            act = nc.scalar
            # blocks 0/1 ramp with 1024-col units sharing one o tile per
            # 2048-col half (single merged store - fewer completion
            # semaphores for the end barrier to poll).
            # PE's very first unit is 512 cols (PE per-op overhead is
            # negligible, so the earlier start shifts its whole packed
            # stream - and the kernel end - ~0.8 us earlier)
            o1 = p_unit(1, 0, 512, flush=False)
            p_unit(1, 512, 512, o=o1, o_off=512, flush=False)
            o0 = d_unit(0, 0, Q, flush=False)
            p_unit(1, Q, Q, o=o1, o_off=Q)
            d_unit(0, Q, Q, o=o0, o_off=Q)
            stream = [
                (p_unit, 1, CH, CH, None), (d_unit, 0, CH, CH, None),
                (p_unit, 3, 0, CH, None), (p_unit, 3, CH, CH, None),
                (d_unit, 2, 0, CH, None), (p_unit, 5, 0, CH, None),
                (p_unit, 5, CH, CH, None), (d_unit, 2, CH, CH, None),
                (p_unit, 6, 0, CH, None), (d_unit, 4, 0, CH, None),
                (p_unit, 6, CH, CH, None), (p_unit, 7, 0, CH, None),
                (d_unit, 4, CH, CH, None), (p_unit, 7, CH, Q, None),
                (p_unit, 7, CH + Q, Q, act),
            ]
            for fn, blk, t0, tl, se in stream:
                fn(blk, t0, tl, se)
    nc.compile()
    return nc


def _shard_inputs(x, w):
    in_maps = []
    for core in range(N_CORES):
        b, half = divmod(core, 2)
        d0 = half * DH
        xt = np.zeros((DH, ROWW), dtype=np.float16)
        xt[:, PAD : PAD + L] = x[b, :, d0 : d0 + DH].T.astype(np.float16)
        # w rows for this shard, rearranged so partition p holds the K
        # weights of channel blk*128 + p at free cols [blk*K, blk*K + K)
        w_sh = w[d0 : d0 + DH].reshape(NBLK, 128, K)
        wt = (
            w_sh.transpose(1, 0, 2).reshape(128, NBLK * K).astype(np.float32)
        )
        # diag blocks for the PE path (P_BLKS only)
        wdv = np.zeros((128, len(P_BLKS), K, 128), dtype=np.float16)
        idx = np.arange(128)
        wdv[idx, :, :, idx] = w_sh[P_BLKS].transpose(1, 0, 2).astype(np.float16)
        in_maps.append(
            {
                "xt": np.ascontiguousarray(xt),
                "wt": np.ascontiguousarray(wt),
                "wd": np.ascontiguousarray(
                    wdv.reshape(128, len(P_BLKS) * K * 128)
                ),
            }
        )
    return in_maps


def kernel(x, w):
    x = np.asarray(x, dtype=np.float32)
    w = np.asarray(w, dtype=np.float32)
    assert x.shape == (B, L, D) and w.shape == (D, K)

    if "nc" not in _cache:
        _cache["nc"] = _build_bass()
    nc = _cache["nc"]

    in_maps = _shard_inputs(x, w)
    res = None
    for attempt in range(3):
        try:
            res = run_bass_kernel_spmd(nc, in_maps, core_ids=list(range(N_CORES)))
            break
        except Exception:
            if attempt == 2:
                raise
    _cache["last_results"] = res

    out = np.empty((B, L, D), dtype=np.float32)
    for core in range(N_CORES):
        b, half = divmod(core, 2)
        d0 = half * DH
        out[b, :, d0 : d0 + DH] = res.results[core]["ot"].T.astype(np.float32)
    return out
